# revision 5
# baseline (speedup 1.0000x reference)
import sys

sys.path.insert(0, "/opt/trn_rl_repo")
import numpy as np
import concourse.bacc as bacc
import concourse.bass as bass
import concourse.mybir as mybir
import concourse.tile as tile
from concourse import bass_utils

F32 = mybir.dt.float32
F32R = mybir.dt.float32r

B = 512          # batch
S = 16384        # state size = 128*128
N_CORES = 8
RPC = B // N_CORES   # 64 rows per core
CHUNKS = 16
RPCH = RPC // CHUNKS  # 4 rows per chunk
CW = RPCH * 128       # 512 chunk width

TRACE = False
LAST_RESULT = None

_prog_cache = {}


def _hadamard128():
    idx = np.arange(128)
    m = idx[:, None] & idx[None, :]
    par = np.zeros_like(m)
    for b in range(7):
        par ^= (m >> b) & 1
    return np.where(par == 0, 1.0, -1.0)


def _bits7():
    # BITS7[q, i] = bit (6-q) of i
    q = np.arange(7)
    i = np.arange(128)
    return ((i[None, :] >> (6 - q)[:, None]) & 1).astype(np.float64)


def _build_program():
    nc = bacc.Bacc("TRN2", target_bir_lowering=False, debug=False)
    d_xre = nc.dram_tensor("xre", [RPC, S], F32R, kind="ExternalInput").ap()
    d_xim = nc.dram_tensor("xim", [RPC, S], F32R, kind="ExternalInput").ap()
    d_h = nc.dram_tensor("h", [128, 128], F32R, kind="ExternalInput").ap()
    d_hs = nc.dram_tensor("hs", [128, 128], F32R, kind="ExternalInput").ap()
    d_id = nc.dram_tensor("ident", [128, 128], F32, kind="ExternalInput").ap()
    d_lre = nc.dram_tensor("lre", [8, CHUNKS * 128], F32R, kind="ExternalInput").ap()
    d_lim = nc.dram_tensor("lim", [8, CHUNKS * 128], F32R, kind="ExternalInput").ap()
    d_ru = nc.dram_tensor("ru", [8, CHUNKS * CW], F32R, kind="ExternalInput").ap()
    d_out = nc.dram_tensor("out", [RPC, S, 2], F32, kind="ExternalOutput").ap()

    with tile.TileContext(nc) as tc:
        with tc.tile_pool(name="const", bufs=1) as cp, \
             tc.tile_pool(name="big", bufs=1) as bigp, \
             tc.tile_pool(name="io", bufs=3) as iop, \
             tc.tile_pool(name="work", bufs=2) as wp, \
             tc.tile_pool(name="ps", bufs=8, space=bass.MemorySpace.PSUM) as psp:

            t_h = cp.tile([128, 128], F32R, name="t_h")
            t_hs = cp.tile([128, 128], F32R, name="t_hs")
            t_id = cp.tile([128, 128], F32, name="t_id")
            t_lre = cp.tile([8, CHUNKS * 128], F32R, name="t_lre")
            t_lim = cp.tile([8, CHUNKS * 128], F32R, name="t_lim")
            for t, d in [(t_h, d_h), (t_hs, d_hs), (t_id, d_id),
                         (t_lre, d_lre), (t_lim, d_lim)]:
                nc.sync.dma_start(t[:], d)

            # A^T storage: [j', (r i')] packed by chunk
            t_are = bigp.tile([128, RPC * 128], F32, name="t_are")
            t_aim = bigp.tile([128, RPC * 128], F32, name="t_aim")

            def ps_tile():
                return psp.tile([128, CW], F32, name="ps", tag="ps")

            # ---------- stage A: A^T = (2^-7 H X H)^T per r-block ----------
            for c in range(CHUNKS):
                cs = slice(c * CW, (c + 1) * CW)
                rs = slice(c * RPCH, (c + 1) * RPCH)
                t_xre = iop.tile([128, CW], F32R, name="t_xre")
                t_xim = iop.tile([128, CW], F32R, name="t_xim")
                nc.sync.dma_start(
                    t_xre[:], d_xre[rs, :].rearrange("r (i j) -> i r j", i=128, j=128))
                nc.sync.dma_start(
                    t_xim[:], d_xim[rs, :].rearrange("r (i j) -> i r j", i=128, j=128))

                p1re = ps_tile()
                nc.tensor.matmul(p1re[:], t_h[:], t_xre[:], start=True, stop=True)
                p1im = ps_tile()
                nc.tensor.matmul(p1im[:], t_h[:], t_xim[:], start=True, stop=True)
                s_u_re = wp.tile([128, CW], F32, name="s_u_re")
                s_u_im = wp.tile([128, CW], F32, name="s_u_im")
                nc.scalar.copy(s_u_re[:], p1re[:])
                nc.scalar.copy(s_u_im[:], p1im[:])
                p2re = ps_tile()
                p2im = ps_tile()
                for b in range(RPCH):
                    bs = slice(b * 128, (b + 1) * 128)
                    nc.tensor.transpose(p2re[:, bs], s_u_re[:, bs], t_id[:])
                    nc.tensor.transpose(p2im[:, bs], s_u_im[:, bs], t_id[:])
                s_ut_re = wp.tile([128, CW], F32R, name="s_ut_re")
                s_ut_im = wp.tile([128, CW], F32R, name="s_ut_im")
                nc.vector.tensor_copy(s_ut_re[:], p2re[:])
                nc.vector.tensor_copy(s_ut_im[:], p2im[:])
                p3re = ps_tile()
                nc.tensor.matmul(p3re[:], t_hs[:], s_ut_re[:], start=True, stop=True)
                p3im = ps_tile()
                nc.tensor.matmul(p3im[:], t_hs[:], s_ut_im[:], start=True, stop=True)
                nc.scalar.copy(t_are[:, cs], p3re[:])
                nc.scalar.copy(t_aim[:, cs], p3im[:])

            # ---------- stage B: B^T = E (x) A^T, Y = 2^-7 H B H ----------
            for c in range(CHUNKS):
                cs = slice(c * CW, (c + 1) * CW)
                rs = slice(c * RPCH, (c + 1) * RPCH)
                ls = slice(c * 128, (c + 1) * 128)
                t_ruc = iop.tile([8, CW], F32R, name="t_ruc")
                nc.sync.dma_start(t_ruc[:], d_ru[:, cs])
                pere = ps_tile()
                nc.tensor.matmul(pere[:], t_lre[:, ls], t_ruc[:],
                                 start=True, stop=True)
                peim = ps_tile()
                nc.tensor.matmul(peim[:], t_lim[:, ls], t_ruc[:],
                                 start=True, stop=True)
                e_re = wp.tile([128, CW], F32, name="e_re")
                e_im = wp.tile([128, CW], F32, name="e_im")
                nc.vector.tensor_copy(e_re[:], pere[:])
                nc.vector.tensor_copy(e_im[:], peim[:])

                P1 = wp.tile([128, CW], F32, name="P1")
                P2 = wp.tile([128, CW], F32, name="P2")
                P3 = wp.tile([128, CW], F32, name="P3")
                P4 = wp.tile([128, CW], F32, name="P4")
                nc.gpsimd.tensor_mul(P1[:], t_are[:, cs], e_re[:])
                nc.gpsimd.tensor_mul(P2[:], t_aim[:, cs], e_im[:])
                nc.gpsimd.tensor_mul(P3[:], t_are[:, cs], e_im[:])
                nc.gpsimd.tensor_mul(P4[:], t_aim[:, cs], e_re[:])
                b_re = wp.tile([128, CW], F32R, name="b_re")
                b_im = wp.tile([128, CW], F32R, name="b_im")
                nc.gpsimd.tensor_sub(b_re[:], P1[:], P2[:])
                nc.gpsimd.tensor_add(b_im[:], P3[:], P4[:])

                p4re = ps_tile()
                nc.tensor.matmul(p4re[:], t_h[:], b_re[:], start=True, stop=True)
                p4im = ps_tile()
                nc.tensor.matmul(p4im[:], t_h[:], b_im[:], start=True, stop=True)
                s_d_re = wp.tile([128, CW], F32, name="s_d_re")
                s_d_im = wp.tile([128, CW], F32, name="s_d_im")
                nc.scalar.copy(s_d_re[:], p4re[:])
                nc.scalar.copy(s_d_im[:], p4im[:])
                p5re = ps_tile()
                p5im = ps_tile()
                for b in range(RPCH):
                    bs = slice(b * 128, (b + 1) * 128)
                    nc.tensor.transpose(p5re[:, bs], s_d_re[:, bs], t_id[:])
                    nc.tensor.transpose(p5im[:, bs], s_d_im[:, bs], t_id[:])
                s_w_re = wp.tile([128, CW], F32R, name="s_w_re")
                s_w_im = wp.tile([128, CW], F32R, name="s_w_im")
                nc.vector.tensor_copy(s_w_re[:], p5re[:])
                nc.vector.tensor_copy(s_w_im[:], p5im[:])
                p6re = ps_tile()
                nc.tensor.matmul(p6re[:], t_hs[:], s_w_re[:], start=True, stop=True)
                p6im = ps_tile()
                nc.tensor.matmul(p6im[:], t_hs[:], s_w_im[:], start=True, stop=True)

                t_out = wp.tile([128, CW, 2], F32, name="t_out")
                nc.scalar.copy(t_out[:, :, 0], p6re[:])
                nc.scalar.copy(t_out[:, :, 1], p6im[:])
                nc.scalar.dma_start(
                    d_out[rs, :, :].rearrange("r (i j) two -> i r j two",
                                              i=128, j=128),
                    t_out[:].rearrange("p a two -> p (a two)"))

    nc.compile()
    return nc


def _host_phase_tables(thetas_core):
    th = thetas_core.astype(np.float64)
    bits = _bits7()
    Pi = 0.5 * (th[:, 0:7] @ bits)    # [64, 128]
    Pj = 0.5 * (th[:, 7:14] @ bits)
    u_re, u_im = np.cos(Pi), -np.sin(Pi)
    v_re, v_im = np.cos(Pj), -np.sin(Pj)
    ure = u_re.reshape(CHUNKS, RPCH, 128)
    uim = u_im.reshape(CHUNKS, RPCH, 128)
    vre = v_re.reshape(CHUNKS, RPCH, 128)
    vim = v_im.reshape(CHUNKS, RPCH, 128)
    ru = np.zeros((8, CHUNKS, RPCH, 128), np.float32)
    lre = np.zeros((8, CHUNKS, 128), np.float32)
    lim_ = np.zeros((8, CHUNKS, 128), np.float32)
    for rl in range(RPCH):
        ru[2 * rl + 0, :, rl, :] = ure[:, rl, :]
        ru[2 * rl + 1, :, rl, :] = uim[:, rl, :]
        lre[2 * rl + 0] = vre[:, rl, :]
        lre[2 * rl + 1] = -vim[:, rl, :]
        lim_[2 * rl + 0] = vim[:, rl, :]
        lim_[2 * rl + 1] = vre[:, rl, :]
    return (ru.reshape(8, CHUNKS * CW), lre.reshape(8, CHUNKS * 128),
            lim_.reshape(8, CHUNKS * 128))


def kernel(phi_real, phi_imag, thetas):
    global LAST_RESULT
    phi_real = np.ascontiguousarray(np.asarray(phi_real, dtype=np.float32))
    phi_imag = np.ascontiguousarray(np.asarray(phi_imag, dtype=np.float32))
    thetas = np.asarray(thetas, dtype=np.float32)

    if "nc" not in _prog_cache:
        _prog_cache["nc"] = _build_program()
    nc = _prog_cache["nc"]

    H = _hadamard128()
    h = H.astype(np.float32)
    hs = (H * (2.0 ** -7)).astype(np.float32)
    ident = np.eye(128, dtype=np.float32)

    in_maps = []
    for k in range(N_CORES):
        rows = slice(k * RPC, (k + 1) * RPC)
        ru, lre, lim_ = _host_phase_tables(thetas[rows])
        in_maps.append({
            "xre": phi_real[rows],
            "xim": phi_imag[rows],
            "h": h,
            "hs": hs,
            "ident": ident,
            "lre": lre,
            "lim": lim_,
            "ru": ru,
        })

    res = bass_utils.run_bass_kernel_spmd(
        nc, in_maps, core_ids=list(range(N_CORES)), trace=TRACE)
    LAST_RESULT = res

    out = np.empty((B, S), dtype=np.complex64)
    for k in range(N_CORES):
        o = np.ascontiguousarray(res.results[k]["out"])  # [64, 16384, 2] f32
        out[k * RPC:(k + 1) * RPC] = o[..., 0] + 1j * o[..., 1]
    return out
